# revision 7
# baseline (speedup 1.0000x reference)
"""Trainium2 Bass kernel for nn_Attend_62534723830373.

Reference computation (note: q is UNUSED by the reference):
    scores = einsum('bhid,bhjd->bhij', k, v) * (1/sqrt(128))
    scores = causal_mask(scores)            # strictly-upper masked
    attn   = softmax(scores, axis=-1)
    out    = einsum('bhij,bhjd->bhid', attn, v)

Shapes: [b=2, h=16, s=2048, d=128] fp32. b*h = 32 head-slices sharded
4-per-core across 8 NeuronCores (data/head parallel, no collectives).

Design notes (evolution from the first working version):
  - bf16 matmul operands (fp16 measured ~25% slower per matmul on this
    PE; bf16 numerics recovered via the tricks below).
  - V is cast-loaded directly into its [V | ones] layout (strided DMA
    dest); the ones column makes column 128 of each MM2 accumulator the
    softmax denominator for free.
  - All exp paths subtract a constant C0 inside the exponent
    (softmax-shift invariant) so fp8e4 weights stay far from the
    saturation/NaN range.
  - exp is split three ways to unbottleneck the scalar engine:
      ACT: true exp activation (bf16 out for diagonal pairs, fp8e4 out
           for off-diagonal "full" pairs)
      DVE: Schraudolph bit-trick - i16 = rint(s*A + B) IS the bf16 bit
           pattern of ~exp(SCALE*s - C0) (+-3.3% max rel err); the fp8
           variant writes uint8 e4m3 bits (saturation at 0 harmlessly
           flushes weights < ~8e-3 to +0).
    A greedy cost model balances the engines. Chunk 0 (rows with < 512
    softmax terms, where a 3% weight error would show) always uses ACT
    bf16; later rows average over >= 512 terms so the sawtooth and fp8
    quantization errors wash out (simulated ~4e-3 end-to-end vs the
    2e-2 budget).
  - MM2 for full pairs runs as fp8e4 DoubleRow matmuls: two j-blocks
    (K=256) per pass at the fp8 2x rate, halving MM2 PE time for ~70%
    of the work. Diagonal pairs stay bf16 (they carry the causal mask).
  - Chunk streams of head PAIRS are interleaved (h, h+1 alternate per
    chunk) so one head's thin early chunks overlap the other's dense
    ones - keeps the PE busy enough that the HAM clock gate stays at
    2.4 GHz (it re-throttled to 1.2 GHz during sparse phases before).
  - Head 0/1 loads are issued before the constant setup so DMA runs
    under it; ~12 dummy matmuls warm the HAM gate during the first DMA.
  - Per-chunk PE transposes write 8 blocks (K and V) into one PSUM
    tile, drained by two 512-wide 2x-mode DVE copies.
  - Epilogue: accumulators drain to SBUF bf16 (two copies), one batched
    reciprocal of the 4 denominators, then 4 scalar-muls into the f32
    output tile.

kernel(**inputs) takes FULL unsharded inputs and returns the FULL output.
"""

import numpy as np

B, H, S, D = 2, 16, 2048, 128
N_CORES = 8
HPC = (B * H) // N_CORES  # heads per core = 4
NB = S // 128             # 16 j/i blocks per head
NCH = S // 512            # 4 i-chunks per head
SCALE = 0.08838834764831845

C0 = 1.5  # constant subtracted inside every exp (softmax-invariant)

# Schraudolph bf16-bits exp: i16 = rint(s_raw*A + B) viewed as bf16
# approximates exp(SCALE*s_raw - C0), max rel err 3.3% (C=5.6
# calibrated numerically; rounding-mode uncertainty absorbed by C).
EXP_A = float(128.0 * SCALE * np.log2(np.e))
EXP_B = 16250.4 - float(128.0 * C0 * np.log2(np.e))
# fp8e4 variant: uint8 = rint(s_raw*A8 + B8) is the e4m3 bit pattern.
EXP_A8 = float(8.0 * SCALE * np.log2(np.e))
EXP_B8 = 56.0 - 0.35 - float(8.0 * C0 * np.log2(np.e))

# engine-balance cost model (ns): greedy chooses the cheaper engine for
# each exp pair given projected busy time. Offsets account for the
# engines' other duties (DVE: transpose drains + masks + epilogue).
DVE_OFFSET_NS = 33000.0
ACT_OFFSET_NS = 3000.0

FP8_MM2 = True    # fp8e4 DoubleRow MM2 on full (off-diagonal) pairs
INTERLEAVE = True  # interleave chunk streams of head pairs
WARM_MM = 12       # dummy 512-wide matmuls to warm the HAM clock gate

_CACHED_NC = None


def _build_nc():
    import concourse.bass as bass
    import concourse.mybir as mybir
    import concourse.tile as tile
    from concourse import bacc
    from concourse.masks import make_identity, make_upper_triangular
    from contextlib import ExitStack

    f32 = mybir.dt.float32
    bf16 = mybir.dt.bfloat16
    i16 = mybir.dt.int16
    u8 = mybir.dt.uint8
    f8e4 = mybir.dt.float8e4
    Exp = mybir.ActivationFunctionType.Exp
    mult = mybir.AluOpType.mult
    add = mybir.AluOpType.add
    DoubleRow = mybir.MatmulPerfMode.DoubleRow

    nc = bacc.Bacc("TRN2", num_devices=N_CORES, debug=False)
    kd = nc.dram_tensor("k", [HPC, S, D], f32, kind="ExternalInput")
    vd = nc.dram_tensor("v", [HPC, S, D], f32, kind="ExternalInput")
    od = nc.dram_tensor("out", [HPC, S, D], f32, kind="ExternalOutput")

    # greedy ACT/DVE exp assignment, deterministic at build time
    act_ns, dve_ns = ACT_OFFSET_NS, DVE_OFFSET_NS
    exp_engine = {}
    for h in range(HPC):
        for ci in range(NCH):
            i0b = 4 * ci
            iend = (i0b + 4) * 128
            for bja in range(0, i0b + 4, 2):
                w = (iend - max(i0b, bja) * 128) + (iend - max(i0b, bja + 1) * 128)
                ca, cd = (172 + w) / 1.2, (120 + w) / 0.96
                if ci == 0 or act_ns + ca <= dve_ns + cd:
                    exp_engine[(h, ci, bja)] = "ACT"
                    act_ns += ca
                else:
                    exp_engine[(h, ci, bja)] = "DVE"
                    dve_ns += cd

    with tile.TileContext(nc) as tc, ExitStack() as ctx:
        const = ctx.enter_context(tc.tile_pool(name="const", bufs=1))
        loadp = ctx.enter_context(tc.tile_pool(name="load", bufs=3))
        ktp = ctx.enter_context(tc.tile_pool(name="kt", bufs=2))
        expp = ctx.enter_context(tc.tile_pool(name="expp", bufs=6))
        exp8p = ctx.enter_context(tc.tile_pool(name="exp8p", bufs=6))
        outp = ctx.enter_context(tc.tile_pool(name="outp", bufs=2))
        epip = ctx.enter_context(tc.tile_pool(name="epi", bufs=2))
        smallp = ctx.enter_context(tc.tile_pool(name="small", bufs=4))
        ps_pool = ctx.enter_context(tc.tile_pool(name="ps", bufs=2, space="PSUM"))
        pt_pool = ctx.enter_context(tc.tile_pool(name="pt", bufs=1, space="PSUM"))
        po_pool = ctx.enter_context(tc.tile_pool(name="po", bufs=3, space="PSUM"))

        # ---- prefetch head 0/1 loads before the const setup ----
        load_tiles = {}

        def issue_loads(h):
            knat = loadp.tile([128, NB, 128], bf16, tag="knat", name=f"knat_{h}")
            vones = loadp.tile([128, NB, 129], bf16, tag="vones", name=f"vones_{h}")
            vones8 = loadp.tile([128, 12, 129], f8e4, tag="vones8", name=f"vones8_{h}")
            kview = kd.ap()[h].rearrange("(n p) d -> p n d", p=128)
            vview = vd.ap()[h].rearrange("(n p) d -> p n d", p=128)
            nc.gpsimd.memset(vones[:, :, 128], 1.0)
            if FP8_MM2:
                nc.gpsimd.memset(vones8[:, :, 128], 1.0)
            for c0, c1 in ((0, 4), (4, 16)):
                sl = slice(c0, c1)
                nc.gpsimd.dma_start(knat[:, sl, :], kview[:, sl, :])
                nc.gpsimd.dma_start(vones[:, sl, 0:128], vview[:, sl, :])
                if FP8_MM2 and c0 < 12:
                    sl8 = slice(c0, min(c1, 12))
                    nc.gpsimd.dma_start(vones8[:, sl8, 0:128], vview[:, sl8, :])
            load_tiles[h] = (knat, vones, vones8)

        issue_loads(0)
        if HPC > 1:
            issue_loads(1)

        trimask = const.tile([128, 128], bf16, tag="trimask")
        make_upper_triangular(nc, trimask[:, :], val=1.0, diag=True)
        ident16 = const.tile([128, 128], bf16, tag="ident16")
        make_identity(nc, ident16[:, :])
        warm_src = const.tile([128, 512], bf16, tag="warm_src")
        nc.vector.memset(warm_src[:, :], 0.0)
        warmf = const.tile([128, 1], f32, tag="warmf")
        nc.vector.memset(warmf[:, :], 1.0)
        biasc0 = const.tile([128, 1], f32, tag="biasc0")
        nc.vector.memset(biasc0[:, :], -C0)
        # warmup exp so ACT's one-time table load happens during startup
        warm = const.tile([128, 1], f32, tag="warm")
        nc.scalar.activation(warm[:, :], warmf[:, :], Exp, scale=SCALE)
        # dummy matmuls: keep the PE busy during the initial DMA wait so
        # the HAM activity monitor ungates the 2.4 GHz clock early
        for r in range(WARM_MM):
            pw = ps_pool.tile([128, 1024], f32, tag="ps", name=f"warmmm_{r}")
            nc.tensor.matmul(
                pw[:, 0:512], ident16[:, :], warm_src[:, :], start=True, stop=True
            )

        # per-head persistent state across the interleaved chunk walk
        KT3s, VT3s, out_sbs = {}, {}, {}

        def head_setup(h):
            if h not in load_tiles:
                issue_loads(h)
            KT3s[h] = ktp.tile([128, NB, 128], bf16, tag="KT", name=f"KT_{h}")
            VT3s[h] = ktp.tile([128, NB, 128], bf16, tag="VT", name=f"VT_{h}")
            out_sbs[h] = outp.tile([128, NB, 128], f32, tag="out_sb", name=f"osb_{h}")

        def do_chunk(h, ci):
            knat, vones, vones8 = load_tiles[h]
            KT = KT3s[h].rearrange("p n d -> p (n d)")
            VT = VT3s[h].rearrange("p n d -> p (n d)")
            out_sb = out_sbs[h]
            i0b = 4 * ci
            iend = (i0b + 4) * 128
            slc = slice(i0b, i0b + 4)
            # JIT per chunk: PE-transpose the chunk's four K and V blocks
            # into one PSUM tile, drain with two 512-wide DVE copies
            pt = pt_pool.tile([128, 8, 128], bf16, tag="pt", name=f"pt_{h}_{ci}")
            for u in range(4):
                bn = i0b + u
                nc.tensor.transpose(pt[:, u, :], knat[:, bn, :], ident16[:, :])
                nc.tensor.transpose(pt[:, 4 + u, :], vones[:, bn, 0:128], ident16[:, :])
            nc.vector.tensor_copy(KT3s[h][:, slc, :], pt[:, 0:4, :])
            nc.vector.tensor_copy(VT3s[h][:, slc, :], pt[:, 4:8, :])

            po = [
                po_pool.tile([128, 258], f32, tag="po", name=f"po_{h}_{ci}_{u}")
                for u in range(2)
            ]

            def po_ap(bi):
                u = bi - i0b
                return po[u // 2][:, (u % 2) * 129 : (u % 2) * 129 + 129]

            # pairs emitted with one-pair lookahead: pair k+1's score
            # matmuls + exp come before pair k's MM2s, so the PE always
            # has score matmuls in flight
            pending = None
            pairs = list(range(0, i0b + 4, 2)) + [None]
            for bja in pairs:
                cur = None
                if bja is not None:
                    bjb = bja + 1
                    full = FP8_MM2 and (bjb < i0b)
                    ista = max(i0b, bja) * 128
                    istb_ = max(i0b, bjb) * 128
                    n1a = iend - ista
                    n1b = iend - istb_
                    ps = ps_pool.tile([128, 1024], f32, tag="ps")
                    nc.tensor.matmul(
                        ps[:, 0:n1a],
                        VT[:, bja * 128 : (bja + 1) * 128],
                        KT[:, ista:iend],
                        start=True,
                        stop=True,
                    )
                    nc.tensor.matmul(
                        ps[:, n1a : n1a + n1b],
                        VT[:, bjb * 128 : (bjb + 1) * 128],
                        KT[:, istb_:iend],
                        start=True,
                        stop=True,
                    )
                    wtot = n1a + n1b
                    on_act = exp_engine[(h, ci, bja)] == "ACT"
                    if full:
                        # off-diagonal pair: fp8e4 weights for DoubleRow MM2
                        ex8 = exp8p.tile([128, 1024], u8, tag="ex8")
                        if on_act:
                            nc.scalar.activation(
                                ex8[:, 0:wtot].bitcast(f8e4),
                                ps[:, 0:wtot],
                                Exp,
                                bias=biasc0[:, :],
                                scale=SCALE,
                            )
                        else:
                            nc.vector.tensor_scalar(
                                ex8[:, 0:wtot],
                                ps[:, 0:wtot],
                                EXP_A8,
                                EXP_B8,
                                op0=mult,
                                op1=add,
                            )
                        cur = ("full", bja, ex8)
                    else:
                        ex = expp.tile([128, 1024], i16, tag="ex")
                        if on_act:
                            nc.scalar.activation(
                                ex[:, 0:wtot].bitcast(bf16),
                                ps[:, 0:wtot],
                                Exp,
                                bias=biasc0[:, :],
                                scale=SCALE,
                            )
                        else:
                            nc.vector.tensor_scalar(
                                ex[:, 0:wtot],
                                ps[:, 0:wtot],
                                EXP_A,
                                EXP_B,
                                op0=mult,
                                op1=add,
                            )
                        if bja >= i0b:
                            # diagonal block: zero the masked (j > i) triangle
                            v = ex[:, 0:128].bitcast(bf16)
                            nc.vector.tensor_tensor(v, v, trimask[:, :], op=mult)
                        if bjb >= i0b:
                            v = ex[:, n1a : n1a + 128].bitcast(bf16)
                            nc.vector.tensor_tensor(v, v, trimask[:, :], op=mult)
                        cur = ("diag", (bja, ista, 0), (bjb, istb_, n1a), ex)
                if pending is not None:
                    if pending[0] == "full":
                        _, pbja, pex8 = pending
                        ex3 = pex8.rearrange("p (t w) -> p t w", t=2)
                        for bi in range(i0b, i0b + 4):
                            c0_ = (bi - i0b) * 128
                            nc.tensor.matmul(
                                po_ap(bi),
                                ex3[:, :, c0_ : c0_ + 128].bitcast(f8e4),
                                vones8[:, pbja : pbja + 2, :],
                                start=(pbja == 0 and (bi - i0b) % 2 == 0),
                                stop=False,
                                perf_mode=DoubleRow,
                                skip_group_check=True,
                            )
                    else:
                        _, pa, pb, pex = pending
                        for bj, ist, off in (pa, pb):
                            for bi in range(ist // 128, i0b + 4):
                                c0_ = off + bi * 128 - ist
                                nc.tensor.matmul(
                                    po_ap(bi),
                                    pex[:, c0_ : c0_ + 128].bitcast(bf16),
                                    vones[:, bj, :],
                                    start=(bj == 0 and (bi - i0b) % 2 == 0),
                                    stop=(bj == bi and (bi - i0b) % 2 == 1),
                                    skip_group_check=True,
                                )
                pending = cur

            # epilogue: drain both accumulator banks to SBUF bf16, one
            # batched reciprocal of the 4 denominators, then
            # out = num * (1/den) per block
            nsb = epip.tile([128, 2, 258], bf16, tag="nsb")
            nc.vector.tensor_copy(nsb[:, 0, :], po[0][:, :])
            nc.vector.tensor_copy(nsb[:, 1, :], po[1][:, :])
            rc = smallp.tile([128, 4], f32, tag="rc")
            den = nsb.rearrange("p u (v c) -> p (u v) c", v=2)[:, :, 128]
            nc.vector.reciprocal(rc[:, :], den)
            for u in range(4):
                bi = i0b + u
                nc.vector.tensor_scalar_mul(
                    out_sb[:, bi, :],
                    nsb[:, u // 2, (u % 2) * 129 : (u % 2) * 129 + 128],
                    rc[:, u : u + 1],
                )
            nc.sync.dma_start(
                od.ap()[h].rearrange("(n p) d -> p n d", p=128)[:, slc, :],
                out_sb[:, slc, :],
            )

        if INTERLEAVE:
            for h0 in range(0, HPC, 2):
                hs = [h0] + ([h0 + 1] if h0 + 1 < HPC else [])
                for h in hs:
                    head_setup(h)
                for ci in range(NCH):
                    for h in hs:
                        do_chunk(h, ci)
        else:
            for h in range(HPC):
                head_setup(h)
                for ci in range(NCH):
                    do_chunk(h, ci)

    nc.finalize()
    return nc


def _get_nc():
    global _CACHED_NC
    if _CACHED_NC is None:
        _CACHED_NC = _build_nc()
    return _CACHED_NC


def run_sharded(k, v, trace=False):
    """k, v: [B*H, S, D] fp32. Returns (out [B*H, S, D], BassKernelResults)."""
    from concourse import bass_utils

    nc = _get_nc()
    in_maps = [
        {
            "k": np.ascontiguousarray(k[c * HPC : (c + 1) * HPC]),
            "v": np.ascontiguousarray(v[c * HPC : (c + 1) * HPC]),
        }
        for c in range(N_CORES)
    ]
    res = bass_utils.run_bass_kernel_spmd(
        nc, in_maps, core_ids=list(range(N_CORES)), trace=trace
    )
    out = np.concatenate([res.results[c]["out"] for c in range(N_CORES)], axis=0)
    return out, res


def kernel(q, k, v):
    k = np.asarray(k, dtype=np.float32).reshape(B * H, S, D)
    v = np.asarray(v, dtype=np.float32).reshape(B * H, S, D)
    out, _ = run_sharded(k, v, trace=False)
    return out.reshape(B, H, S, D)


# revision 9
# speedup vs baseline: 1.0138x; 1.0138x over previous
"""Trainium2 Bass kernel for nn_Attend_62534723830373.

Reference computation (note: q is UNUSED by the reference):
    scores = einsum('bhid,bhjd->bhij', k, v) * (1/sqrt(128))
    scores = causal_mask(scores)            # strictly-upper masked
    attn   = softmax(scores, axis=-1)
    out    = einsum('bhij,bhjd->bhid', attn, v)

Shapes: [b=2, h=16, s=2048, d=128] fp32. b*h = 32 head-slices sharded
4-per-core across 8 NeuronCores (data/head parallel, no collectives).

Design notes (evolution from the first working version):
  - bf16 matmul operands (fp16 measured ~25% slower per matmul on this
    PE; bf16 numerics recovered via the tricks below).
  - V is cast-loaded directly into its [V | ones] layout (strided DMA
    dest); the ones column makes column 128 of each MM2 accumulator the
    softmax denominator for free.
  - All exp paths subtract a constant C0 inside the exponent
    (softmax-shift invariant) so fp8e4 weights stay far from the
    saturation/NaN range.
  - exp is split three ways to unbottleneck the scalar engine:
      ACT: true exp activation (bf16 out for diagonal pairs, fp8e4 out
           for off-diagonal "full" pairs)
      DVE: Schraudolph bit-trick - i16 = rint(s*A + B) IS the bf16 bit
           pattern of ~exp(SCALE*s - C0) (+-3.3% max rel err); the fp8
           variant writes uint8 e4m3 bits (saturation at 0 harmlessly
           flushes weights < ~8e-3 to +0).
    A greedy cost model balances the engines. Chunk 0 (rows with < 512
    softmax terms, where a 3% weight error would show) always uses ACT
    bf16; later rows average over >= 512 terms so the sawtooth and fp8
    quantization errors wash out (simulated ~4e-3 end-to-end vs the
    2e-2 budget).
  - MM2 for full pairs runs as fp8e4 DoubleRow matmuls: two j-blocks
    (K=256) per pass at the fp8 2x rate, halving MM2 PE time for ~70%
    of the work. Diagonal pairs stay bf16 (they carry the causal mask).
  - Chunk streams of head PAIRS are interleaved (h, h+1 alternate per
    chunk) so one head's thin early chunks overlap the other's dense
    ones - keeps the PE busy enough that the HAM clock gate stays at
    2.4 GHz (it re-throttled to 1.2 GHz during sparse phases before).
  - Head 0/1 loads are issued before the constant setup so DMA runs
    under it; ~12 dummy matmuls warm the HAM gate during the first DMA.
  - Per-chunk PE transposes write 8 blocks (K and V) into one PSUM
    tile, drained by two 512-wide 2x-mode DVE copies.
  - Epilogue: accumulators drain to SBUF bf16 (two copies), one batched
    reciprocal of the 4 denominators, then 4 scalar-muls into the f32
    output tile.

kernel(**inputs) takes FULL unsharded inputs and returns the FULL output.
"""

import numpy as np

B, H, S, D = 2, 16, 2048, 128
N_CORES = 8
HPC = (B * H) // N_CORES  # heads per core = 4
NB = S // 128             # 16 j/i blocks per head
NCH = S // 512            # 4 i-chunks per head
SCALE = 0.08838834764831845

C0 = 1.5  # constant subtracted inside every exp (softmax-invariant)

# Schraudolph bf16-bits exp: i16 = rint(s_raw*A + B) viewed as bf16
# approximates exp(SCALE*s_raw - C0), max rel err 3.3% (C=5.6
# calibrated numerically; rounding-mode uncertainty absorbed by C).
EXP_A = float(128.0 * SCALE * np.log2(np.e))
EXP_B = 16250.4 - float(128.0 * C0 * np.log2(np.e))
# fp8e4 variant: uint8 = rint(s_raw*A8 + B8) is the e4m3 bit pattern.
EXP_A8 = float(8.0 * SCALE * np.log2(np.e))
EXP_B8 = 56.0 - 0.35 - float(8.0 * C0 * np.log2(np.e))

# engine-balance cost model (ns): greedy chooses the cheaper engine for
# each exp pair given projected busy time. Offsets account for the
# engines' other duties (DVE: transpose drains + masks + epilogue).
DVE_OFFSET_NS = 33000.0
ACT_OFFSET_NS = 3000.0

FP8_MM2 = True    # fp8e4 DoubleRow MM2 on full (off-diagonal) pairs
INTERLEAVE = True  # interleave chunk streams of head pairs
WARM_MM = 12       # dummy 512-wide matmuls to warm the HAM clock gate

_CACHED_NC = None


def _build_nc():
    import concourse.bass as bass
    import concourse.mybir as mybir
    import concourse.tile as tile
    from concourse import bacc
    from concourse.masks import make_identity, make_upper_triangular
    from contextlib import ExitStack

    f32 = mybir.dt.float32
    bf16 = mybir.dt.bfloat16
    i16 = mybir.dt.int16
    u8 = mybir.dt.uint8
    f8e4 = mybir.dt.float8e4
    Exp = mybir.ActivationFunctionType.Exp
    mult = mybir.AluOpType.mult
    add = mybir.AluOpType.add
    DoubleRow = mybir.MatmulPerfMode.DoubleRow

    nc = bacc.Bacc("TRN2", num_devices=N_CORES, debug=False)
    kd = nc.dram_tensor("k", [HPC, S, D], f32, kind="ExternalInput")
    vd = nc.dram_tensor("v", [HPC, S, D], f32, kind="ExternalInput")
    od = nc.dram_tensor("out", [HPC, S, D], f32, kind="ExternalOutput")

    # greedy ACT/DVE exp assignment, deterministic at build time
    act_ns, dve_ns = ACT_OFFSET_NS, DVE_OFFSET_NS
    exp_engine = {}
    for h in range(HPC):
        for ci in range(NCH):
            i0b = 4 * ci
            iend = (i0b + 4) * 128
            for bja in range(0, i0b + 4, 2):
                w = (iend - max(i0b, bja) * 128) + (iend - max(i0b, bja + 1) * 128)
                ca, cd = (172 + w) / 1.2, (120 + w) / 0.96
                if ci == 0 or act_ns + ca <= dve_ns + cd:
                    exp_engine[(h, ci, bja)] = "ACT"
                    act_ns += ca
                else:
                    exp_engine[(h, ci, bja)] = "DVE"
                    dve_ns += cd

    with tile.TileContext(nc) as tc, ExitStack() as ctx:
        const = ctx.enter_context(tc.tile_pool(name="const", bufs=1))
        loadp = ctx.enter_context(tc.tile_pool(name="load", bufs=3))
        ktp = ctx.enter_context(tc.tile_pool(name="kt", bufs=2))
        expp = ctx.enter_context(tc.tile_pool(name="expp", bufs=6))
        exp8p = ctx.enter_context(tc.tile_pool(name="exp8p", bufs=6))
        outp = ctx.enter_context(tc.tile_pool(name="outp", bufs=2))
        epip = ctx.enter_context(tc.tile_pool(name="epi", bufs=2))
        smallp = ctx.enter_context(tc.tile_pool(name="small", bufs=4))
        ps_pool = ctx.enter_context(tc.tile_pool(name="ps", bufs=2, space="PSUM"))
        pt_pool = ctx.enter_context(tc.tile_pool(name="pt", bufs=1, space="PSUM"))
        po_pool = ctx.enter_context(tc.tile_pool(name="po", bufs=3, space="PSUM"))

        # each SWDGE dma_start costs ~1.1us of GPSIMD issue time, so the
        # start of the kernel is choreographed: consts first (they gate
        # the PE warmup + transposes), then just the first-chunk blocks
        # of heads 0/1, then the remainders; heads 2+ use one start per
        # tensor, prefetched a full head-pair ahead.
        load_tiles = {}

        def alloc_load(h):
            knat = loadp.tile([128, NB, 128], bf16, tag="knat", name=f"knat_{h}")
            vones = loadp.tile([128, NB, 129], bf16, tag="vones", name=f"vones_{h}")
            vones8 = loadp.tile([128, 12, 129], f8e4, tag="vones8", name=f"vones8_{h}")
            nc.gpsimd.memset(vones[:, :, 128], 1.0)
            if FP8_MM2:
                nc.gpsimd.memset(vones8[:, :, 128], 1.0)
            load_tiles[h] = (knat, vones, vones8)
            return load_tiles[h]

        def issue_loads(h, part):
            knat, vones, vones8 = load_tiles[h]
            kview = kd.ap()[h].rearrange("(n p) d -> p n d", p=128)
            vview = vd.ap()[h].rearrange("(n p) d -> p n d", p=128)
            if part in ("early", "all"):
                sl = slice(0, 4) if part == "early" else slice(0, 16)
                nc.gpsimd.dma_start(knat[:, sl, :], kview[:, sl, :])
                nc.gpsimd.dma_start(vones[:, sl, 0:128], vview[:, sl, :])
            if part == "rest":
                sl = slice(4, 16)
                nc.gpsimd.dma_start(knat[:, sl, :], kview[:, sl, :])
                nc.gpsimd.dma_start(vones[:, sl, 0:128], vview[:, sl, :])
            if part in ("rest", "all") and FP8_MM2:
                nc.gpsimd.dma_start(
                    vones8[:, 0:12, 0:128], vview[:, 0:12, :]
                )

        trimask = const.tile([128, 128], bf16, tag="trimask")
        make_upper_triangular(nc, trimask[:, :], val=1.0, diag=True)
        ident16 = const.tile([128, 128], bf16, tag="ident16")
        make_identity(nc, ident16[:, :])
        warm_src = const.tile([128, 512], bf16, tag="warm_src")
        nc.vector.memset(warm_src[:, :], 0.0)
        warmf = const.tile([128, 1], f32, tag="warmf")
        nc.vector.memset(warmf[:, :], 1.0)
        biasc0 = const.tile([128, 1], f32, tag="biasc0")
        nc.vector.memset(biasc0[:, :], -C0)
        # warmup exp so ACT's one-time table load happens during startup
        warm = const.tile([128, 1], f32, tag="warm")
        nc.scalar.activation(warm[:, :], warmf[:, :], Exp, scale=SCALE)
        # dummy matmuls: keep the PE busy during the initial DMA wait so
        # the HAM activity monitor ungates the 2.4 GHz clock early
        for r in range(WARM_MM):
            pw = ps_pool.tile([128, 1024], f32, tag="ps", name=f"warmmm_{r}")
            nc.tensor.matmul(
                pw[:, 0:512], ident16[:, :], warm_src[:, :], start=True, stop=True
            )

        for h in range(min(2, HPC)):
            alloc_load(h)
        for h in range(min(2, HPC)):
            issue_loads(h, "early")
        for h in range(min(2, HPC)):
            issue_loads(h, "rest")

        # per-head persistent state across the interleaved chunk walk
        KT3s, VT3s, out_sbs = {}, {}, {}

        def head_setup(h):
            if h not in load_tiles:
                alloc_load(h)
                issue_loads(h, "all")
            KT3s[h] = ktp.tile([128, NB, 128], bf16, tag="KT", name=f"KT_{h}")
            VT3s[h] = ktp.tile([128, NB, 128], bf16, tag="VT", name=f"VT_{h}")
            out_sbs[h] = outp.tile([128, NB, 128], f32, tag="out_sb", name=f"osb_{h}")

        def do_chunk(h, ci):
            knat, vones, vones8 = load_tiles[h]
            KT = KT3s[h].rearrange("p n d -> p (n d)")
            VT = VT3s[h].rearrange("p n d -> p (n d)")
            out_sb = out_sbs[h]
            i0b = 4 * ci
            iend = (i0b + 4) * 128
            slc = slice(i0b, i0b + 4)
            # JIT per chunk: PE-transpose the chunk's four K and V blocks
            # into one PSUM tile, drain with two 512-wide DVE copies
            pt = pt_pool.tile([128, 8, 128], bf16, tag="pt", name=f"pt_{h}_{ci}")
            for u in range(4):
                bn = i0b + u
                nc.tensor.transpose(pt[:, u, :], knat[:, bn, :], ident16[:, :])
                nc.tensor.transpose(pt[:, 4 + u, :], vones[:, bn, 0:128], ident16[:, :])
            nc.vector.tensor_copy(KT3s[h][:, slc, :], pt[:, 0:4, :])
            nc.vector.tensor_copy(VT3s[h][:, slc, :], pt[:, 4:8, :])

            po = [
                po_pool.tile([128, 258], f32, tag="po", name=f"po_{h}_{ci}_{u}")
                for u in range(2)
            ]

            def po_ap(bi):
                u = bi - i0b
                return po[u // 2][:, (u % 2) * 129 : (u % 2) * 129 + 129]

            # pairs emitted with one-pair lookahead: pair k+1's score
            # matmuls + exp come before pair k's MM2s, so the PE always
            # has score matmuls in flight
            pending = None
            pairs = list(range(0, i0b + 4, 2)) + [None]
            for bja in pairs:
                cur = None
                if bja is not None:
                    bjb = bja + 1
                    full = FP8_MM2 and (bjb < i0b)
                    ista = max(i0b, bja) * 128
                    istb_ = max(i0b, bjb) * 128
                    n1a = iend - ista
                    n1b = iend - istb_
                    ps = ps_pool.tile([128, 1024], f32, tag="ps")
                    nc.tensor.matmul(
                        ps[:, 0:n1a],
                        VT[:, bja * 128 : (bja + 1) * 128],
                        KT[:, ista:iend],
                        start=True,
                        stop=True,
                    )
                    nc.tensor.matmul(
                        ps[:, n1a : n1a + n1b],
                        VT[:, bjb * 128 : (bjb + 1) * 128],
                        KT[:, istb_:iend],
                        start=True,
                        stop=True,
                    )
                    wtot = n1a + n1b
                    on_act = exp_engine[(h, ci, bja)] == "ACT"
                    if full:
                        # off-diagonal pair: fp8e4 weights for DoubleRow MM2
                        ex8 = exp8p.tile([128, 1024], u8, tag="ex8")
                        if on_act:
                            nc.scalar.activation(
                                ex8[:, 0:wtot].bitcast(f8e4),
                                ps[:, 0:wtot],
                                Exp,
                                bias=biasc0[:, :],
                                scale=SCALE,
                            )
                        else:
                            nc.vector.tensor_scalar(
                                ex8[:, 0:wtot],
                                ps[:, 0:wtot],
                                EXP_A8,
                                EXP_B8,
                                op0=mult,
                                op1=add,
                            )
                        cur = ("full", bja, ex8)
                    else:
                        ex = expp.tile([128, 1024], i16, tag="ex")
                        if on_act:
                            nc.scalar.activation(
                                ex[:, 0:wtot].bitcast(bf16),
                                ps[:, 0:wtot],
                                Exp,
                                bias=biasc0[:, :],
                                scale=SCALE,
                            )
                        else:
                            nc.vector.tensor_scalar(
                                ex[:, 0:wtot],
                                ps[:, 0:wtot],
                                EXP_A,
                                EXP_B,
                                op0=mult,
                                op1=add,
                            )
                        if bja >= i0b:
                            # diagonal block: zero the masked (j > i) triangle
                            v = ex[:, 0:128].bitcast(bf16)
                            nc.vector.tensor_tensor(v, v, trimask[:, :], op=mult)
                        if bjb >= i0b:
                            v = ex[:, n1a : n1a + 128].bitcast(bf16)
                            nc.vector.tensor_tensor(v, v, trimask[:, :], op=mult)
                        cur = ("diag", (bja, ista, 0), (bjb, istb_, n1a), ex)
                if pending is not None:
                    if pending[0] == "full":
                        _, pbja, pex8 = pending
                        ex3 = pex8.rearrange("p (t w) -> p t w", t=2)
                        for bi in range(i0b, i0b + 4):
                            c0_ = (bi - i0b) * 128
                            nc.tensor.matmul(
                                po_ap(bi),
                                ex3[:, :, c0_ : c0_ + 128].bitcast(f8e4),
                                vones8[:, pbja : pbja + 2, :],
                                start=(pbja == 0 and (bi - i0b) % 2 == 0),
                                stop=False,
                                perf_mode=DoubleRow,
                                skip_group_check=True,
                            )
                    else:
                        _, pa, pb, pex = pending
                        for bj, ist, off in (pa, pb):
                            for bi in range(ist // 128, i0b + 4):
                                c0_ = off + bi * 128 - ist
                                nc.tensor.matmul(
                                    po_ap(bi),
                                    pex[:, c0_ : c0_ + 128].bitcast(bf16),
                                    vones[:, bj, :],
                                    start=(bj == 0 and (bi - i0b) % 2 == 0),
                                    stop=(bj == bi and (bi - i0b) % 2 == 1),
                                    skip_group_check=True,
                                )
                pending = cur

            # epilogue: drain both accumulator banks to SBUF bf16, one
            # batched reciprocal of the 4 denominators, then
            # out = num * (1/den) per block
            nsb = epip.tile([128, 2, 258], bf16, tag="nsb")
            nc.vector.tensor_copy(nsb[:, 0, :], po[0][:, :])
            nc.vector.tensor_copy(nsb[:, 1, :], po[1][:, :])
            rc = smallp.tile([128, 4], f32, tag="rc")
            den = nsb.rearrange("p u (v c) -> p (u v) c", v=2)[:, :, 128]
            nc.vector.reciprocal(rc[:, :], den)
            for u in range(4):
                bi = i0b + u
                nc.vector.tensor_scalar_mul(
                    out_sb[:, bi, :],
                    nsb[:, u // 2, (u % 2) * 129 : (u % 2) * 129 + 128],
                    rc[:, u : u + 1],
                )
            nc.sync.dma_start(
                od.ap()[h].rearrange("(n p) d -> p n d", p=128)[:, slc, :],
                out_sb[:, slc, :],
            )

        if INTERLEAVE:
            for h0 in range(0, HPC, 2):
                hs = [h0] + ([h0 + 1] if h0 + 1 < HPC else [])
                for h in hs:
                    head_setup(h)
                for ci in range(NCH):
                    for h in hs:
                        do_chunk(h, ci)
        else:
            for h in range(HPC):
                head_setup(h)
                for ci in range(NCH):
                    do_chunk(h, ci)

    nc.finalize()
    return nc


def _get_nc():
    global _CACHED_NC
    if _CACHED_NC is None:
        _CACHED_NC = _build_nc()
    return _CACHED_NC


def run_sharded(k, v, trace=False):
    """k, v: [B*H, S, D] fp32. Returns (out [B*H, S, D], BassKernelResults)."""
    from concourse import bass_utils

    nc = _get_nc()
    in_maps = [
        {
            "k": np.ascontiguousarray(k[c * HPC : (c + 1) * HPC]),
            "v": np.ascontiguousarray(v[c * HPC : (c + 1) * HPC]),
        }
        for c in range(N_CORES)
    ]
    res = bass_utils.run_bass_kernel_spmd(
        nc, in_maps, core_ids=list(range(N_CORES)), trace=trace
    )
    out = np.concatenate([res.results[c]["out"] for c in range(N_CORES)], axis=0)
    return out, res


def kernel(q, k, v):
    k = np.asarray(k, dtype=np.float32).reshape(B * H, S, D)
    v = np.asarray(v, dtype=np.float32).reshape(B * H, S, D)
    out, _ = run_sharded(k, v, trace=False)
    return out.reshape(B, H, S, D)


# revision 10
# speedup vs baseline: 1.1002x; 1.0852x over previous
"""Trainium2 Bass kernel for nn_Attend_62534723830373.

Reference computation (note: q is UNUSED by the reference):
    scores = einsum('bhid,bhjd->bhij', k, v) * (1/sqrt(128))
    scores = causal_mask(scores)            # strictly-upper masked
    attn   = softmax(scores, axis=-1)
    out    = einsum('bhij,bhjd->bhid', attn, v)

Shapes: [b=2, h=16, s=2048, d=128] fp32. b*h = 32 head-slices sharded
4-per-core across 8 NeuronCores (data/head parallel, no collectives).

Per-head dataflow on one core (matmul chain in bf16, fp32 accumulate):
  - SWDGE cast-load K, V (fp32 HBM -> bf16 SBUF, natural layout), the
    first 4 row-blocks in their own chunk so compute starts early. V is
    additionally cast-loaded as fp8e4 in [V | ones] layout for the
    DoubleRow MM2 path below.
  - Just-in-time per 512-wide i-chunk: transpose the chunk's four
    128x128 blocks of K and V on the PE (bf16 transpose + DVE copy) to
    build KT[d, s] / VT[d, s], and assemble [V | ones] (129 cols).
  - For each i-chunk, j-block pairs share one 1024-wide (2-bank) PSUM
    score tile and ONE exp instruction (halves ACT's ~293ns fixed cost
    per instruction), emitted with one-pair lookahead so the PE always
    has score matmuls in flight:
      S^T[j, i] = (VT_blk).T @ KT_slice        (PE, contraction d)
      E = exp(SCALE * S^T - C0)                (ACT, PSUM -> SBUF)
      diag block: E *= upper-tri 0/1 mask      (DVE)
      psum_o[i-blk] += E_slice.T @ [V | 1]     (PE, contraction j)
    The constant C0 inside the exp is softmax-shift invariant; it keeps
    the fp8e4 weights far below the e4m3 saturation point.
    OFF-DIAGONAL ("full") pairs write their weights as fp8e4 and their
    MM2 runs as a DoubleRow matmul: both j-blocks (K=256) in one pass
    at the fp8 2x rate, halving MM2 PE time for ~70% of the MM2 work.
    Diagonal pairs stay bf16 (they carry the causal mask, and their
    rows include the short-softmax rows where fp8 error would show).
    The ones column makes column 128 of each accumulator the softmax
    denominator - numerator and denominator in one accumulation, and
    the denominator sees the same quantized weights so the error
    largely cancels in the normalization. Two accumulators share each
    PSUM bank; since matmul start=True clears has_written bank-wide,
    each bank is a single accumulation group (start only on the bank's
    first write, stop on its last).
  - out = psum_o[:, 0:128] * (1 / psum_o[:, 128])  (DVE recip + mul),
    stored per i-chunk so the final DMA is small.

kernel(**inputs) takes FULL unsharded inputs and returns the FULL output.
"""

import numpy as np

B, H, S, D = 2, 16, 2048, 128
N_CORES = 8
HPC = (B * H) // N_CORES  # heads per core = 4
NB = S // 128             # 16 j/i blocks per head
NCH = S // 512            # 4 i-chunks per head
SCALE = 0.08838834764831845
C0 = 1.5                  # constant subtracted inside every exp

FP8_MM2 = True

_CACHED_NC = None


def _build_nc():
    import concourse.bass as bass
    import concourse.mybir as mybir
    import concourse.tile as tile
    from concourse import bacc
    from concourse.masks import make_identity, make_upper_triangular
    from contextlib import ExitStack

    f32 = mybir.dt.float32
    bf16 = mybir.dt.bfloat16
    u8 = mybir.dt.uint8
    f8e4 = mybir.dt.float8e4
    Exp = mybir.ActivationFunctionType.Exp
    DoubleRow = mybir.MatmulPerfMode.DoubleRow

    nc = bacc.Bacc("TRN2", num_devices=N_CORES, debug=False)
    kd = nc.dram_tensor("k", [HPC, S, D], f32, kind="ExternalInput")
    vd = nc.dram_tensor("v", [HPC, S, D], f32, kind="ExternalInput")
    od = nc.dram_tensor("out", [HPC, S, D], f32, kind="ExternalOutput")

    with tile.TileContext(nc) as tc, ExitStack() as ctx:
        const = ctx.enter_context(tc.tile_pool(name="const", bufs=1))
        loadp = ctx.enter_context(tc.tile_pool(name="load", bufs=2))
        ktp = ctx.enter_context(tc.tile_pool(name="kt", bufs=2))
        expp = ctx.enter_context(tc.tile_pool(name="expp", bufs=4))
        exp8p = ctx.enter_context(tc.tile_pool(name="exp8p", bufs=4))
        outp = ctx.enter_context(tc.tile_pool(name="outp", bufs=2))
        smallp = ctx.enter_context(tc.tile_pool(name="small", bufs=4))
        ps_pool = ctx.enter_context(tc.tile_pool(name="ps", bufs=2, space="PSUM"))
        pt_pool = ctx.enter_context(tc.tile_pool(name="pt", bufs=2, space="PSUM"))
        po_pool = ctx.enter_context(tc.tile_pool(name="po", bufs=2, space="PSUM"))

        trimask_f32 = const.tile([128, 128], f32, tag="trimask_f32")
        make_upper_triangular(nc, trimask_f32[:, :], val=1.0, diag=True)
        trimask = const.tile([128, 128], bf16, tag="trimask")
        nc.vector.tensor_copy(trimask[:, :], trimask_f32[:, :])
        onesf32 = const.tile([128, NB], f32, tag="onesf32")
        nc.gpsimd.memset(onesf32[:, :], 1.0)
        identbf = const.tile([128, 128], bf16, tag="identbf")
        make_identity(nc, identbf[:, :])
        biasc0 = const.tile([128, 1], f32, tag="biasc0")
        nc.vector.memset(biasc0[:, :], -C0)
        # warmup exp so ACT's one-time table load happens during startup
        # instead of on the first real score tile's critical path
        warm = const.tile([128, 1], f32, tag="warm")
        nc.scalar.activation(warm[:, :], onesf32[:, 0:1], Exp, scale=SCALE)

        for h in range(HPC):
            # ---- loads: fp32 HBM -> bf16 SBUF (SWDGE cast), natural ----
            knat = loadp.tile([128, NB, 128], bf16, tag="knat")
            vnat = loadp.tile([128, NB, 128], bf16, tag="vnat")
            vones = loadp.tile([128, NB, 129], bf16, tag="vones")
            vones8 = loadp.tile([128, 12, 129], f8e4, tag="vones8")
            KT3 = ktp.tile([128, NB, 128], bf16, tag="KT")
            VT3 = ktp.tile([128, NB, 128], bf16, tag="VT")
            kview = kd.ap()[h].rearrange("(n p) d -> p n d", p=128)
            vview = vd.ap()[h].rearrange("(n p) d -> p n d", p=128)
            # first 4 blocks in their own chunk so chunk-0 compute can
            # start early (each SWDGE dma_start costs ~1.4us of Q7 issue,
            # so only two chunks per matrix)
            for c0, c1 in ((0, 4), (4, 16)):
                sl = slice(c0, c1)
                nc.gpsimd.dma_start(knat[:, sl, :], kview[:, sl, :])
                nc.gpsimd.dma_start(vnat[:, sl, :], vview[:, sl, :])
            if FP8_MM2:
                nc.gpsimd.memset(vones8[:, :, 128], 1.0)
                nc.gpsimd.dma_start(vones8[:, 0:12, 0:128], vview[:, 0:12, :])
            KT = KT3.rearrange("p n d -> p (n d)")
            VT = VT3.rearrange("p n d -> p (n d)")

            out_sb = outp.tile([128, NB, 128], f32, tag="out_sb")

            # ---- main causal attention loop ----
            for ci in range(NCH):
                i0b = 4 * ci              # first i-block of chunk
                iend = (i0b + 4) * 128
                # just-in-time per chunk: [V | ones] columns and PE
                # transposes (+ DVE copies) for blocks 4ci..4ci+3; spreads
                # the transpose bursts across the head instead of one
                # stall-prone prologue burst
                slc = slice(i0b, i0b + 4)
                nc.vector.tensor_copy(vones[:, slc, 0:128], vnat[:, slc, :])
                nc.vector.tensor_copy(vones[:, slc, 128], onesf32[:, slc])
                for bn in range(i0b, i0b + 4):
                    pstk = pt_pool.tile(
                        [128, 128], bf16, tag="pt", name=f"pstk_{h}_{bn}"
                    )
                    nc.tensor.transpose(pstk[:, :], knat[:, bn, :], identbf[:, :])
                    nc.vector.tensor_copy(KT3[:, bn, :], pstk[:, :])
                    pstv = pt_pool.tile(
                        [128, 128], bf16, tag="pt", name=f"pstv_{h}_{bn}"
                    )
                    nc.tensor.transpose(pstv[:, :], vnat[:, bn, :], identbf[:, :])
                    nc.vector.tensor_copy(VT3[:, bn, :], pstv[:, :])
                po = [
                    po_pool.tile([128, 258], f32, tag="po", name=f"po_{h}_{ci}_{u}")
                    for u in range(2)
                ]

                def po_ap(bi):
                    u = bi - i0b
                    return po[u // 2][:, (u % 2) * 129 : (u % 2) * 129 + 129]

                # pairs are emitted with one-pair lookahead: pair k+1's
                # score matmuls + exp come before pair k's MM2s, so the PE
                # has work while the first MM2 of a chunk waits for the po
                # banks to be freed by the previous chunk's epilogue
                pending = None  # awaiting MM2 emission
                pairs = list(range(0, i0b + 4, 2)) + [None]
                for bja in pairs:
                    cur = None
                    if bja is not None:
                        bjb = bja + 1
                        full = FP8_MM2 and (bjb < i0b)
                        ista = max(i0b, bja) * 128
                        istb_ = max(i0b, bjb) * 128
                        n1a = iend - ista
                        n1b = iend - istb_
                        ps = ps_pool.tile([128, 1024], f32, tag="ps")
                        nc.tensor.matmul(
                            ps[:, 0:n1a],
                            VT[:, bja * 128 : (bja + 1) * 128],
                            KT[:, ista:iend],
                            start=True,
                            stop=True,
                        )
                        nc.tensor.matmul(
                            ps[:, n1a : n1a + n1b],
                            VT[:, bjb * 128 : (bjb + 1) * 128],
                            KT[:, istb_:iend],
                            start=True,
                            stop=True,
                        )
                        if full:
                            # fp8e4 weights for the DoubleRow MM2
                            ex8 = exp8p.tile([128, 1024], u8, tag="ex8")
                            nc.scalar.activation(
                                ex8[:, 0 : n1a + n1b].bitcast(f8e4),
                                ps[:, 0 : n1a + n1b],
                                Exp,
                                bias=biasc0[:, :],
                                scale=SCALE,
                            )
                            cur = ("full", bja, ex8)
                        else:
                            ex = expp.tile([128, 1024], bf16, tag="ex")
                            nc.scalar.activation(
                                ex[:, 0 : n1a + n1b],
                                ps[:, 0 : n1a + n1b],
                                Exp,
                                bias=biasc0[:, :],
                                scale=SCALE,
                            )
                            if bja >= i0b:
                                # diagonal: zero j > i strict lower triangle
                                nc.vector.tensor_mul(
                                    ex[:, 0:128], ex[:, 0:128], trimask[:, :]
                                )
                            if bjb >= i0b:
                                nc.vector.tensor_mul(
                                    ex[:, n1a : n1a + 128],
                                    ex[:, n1a : n1a + 128],
                                    trimask[:, :],
                                )
                            cur = ("diag", (bja, ista, 0), (bjb, istb_, n1a), ex)
                    if pending is not None:
                        if pending[0] == "full":
                            _, pbja, pex8 = pending
                            ex3 = pex8.rearrange("p (t w) -> p t w", t=2)
                            for bi in range(i0b, i0b + 4):
                                c0_ = (bi - i0b) * 128
                                nc.tensor.matmul(
                                    po_ap(bi),
                                    ex3[:, :, c0_ : c0_ + 128].bitcast(f8e4),
                                    vones8[:, pbja : pbja + 2, :],
                                    start=(pbja == 0 and (bi - i0b) % 2 == 0),
                                    stop=False,
                                    perf_mode=DoubleRow,
                                    skip_group_check=True,
                                )
                        else:
                            _, pa, pb, pex = pending
                            for bj, ist, off in (pa, pb):
                                for bi in range(ist // 128, i0b + 4):
                                    c0_ = off + bi * 128 - ist
                                    nc.tensor.matmul(
                                        po_ap(bi),
                                        pex[:, c0_ : c0_ + 128],
                                        vones[:, bj, :],
                                        start=(bj == 0 and (bi - i0b) % 2 == 0),
                                        stop=(bj == bi and (bi - i0b) % 2 == 1),
                                        skip_group_check=True,
                                    )
                    pending = cur
                for u in range(4):
                    bi = i0b + u
                    rc = smallp.tile([128, 1], f32, tag="rc")
                    nc.vector.reciprocal(rc[:, :], po_ap(bi)[:, 128:129])
                    nc.vector.tensor_scalar_mul(
                        out_sb[:, bi, :], po_ap(bi)[:, 0:128], rc[:, :]
                    )
                nc.sync.dma_start(
                    od.ap()[h].rearrange("(n p) d -> p n d", p=128)[
                        :, i0b : i0b + 4, :
                    ],
                    out_sb[:, i0b : i0b + 4, :],
                )

    nc.finalize()
    return nc


def _get_nc():
    global _CACHED_NC
    if _CACHED_NC is None:
        _CACHED_NC = _build_nc()
    return _CACHED_NC


def run_sharded(k, v, trace=False):
    """k, v: [B*H, S, D] fp32. Returns (out [B*H, S, D], BassKernelResults)."""
    from concourse import bass_utils

    nc = _get_nc()
    in_maps = [
        {
            "k": np.ascontiguousarray(k[c * HPC : (c + 1) * HPC]),
            "v": np.ascontiguousarray(v[c * HPC : (c + 1) * HPC]),
        }
        for c in range(N_CORES)
    ]
    res = bass_utils.run_bass_kernel_spmd(
        nc, in_maps, core_ids=list(range(N_CORES)), trace=trace
    )
    out = np.concatenate([res.results[c]["out"] for c in range(N_CORES)], axis=0)
    return out, res


def kernel(q, k, v):
    k = np.asarray(k, dtype=np.float32).reshape(B * H, S, D)
    v = np.asarray(v, dtype=np.float32).reshape(B * H, S, D)
    out, _ = run_sharded(k, v, trace=False)
    return out.reshape(B, H, S, D)


# revision 12
# speedup vs baseline: 1.1420x; 1.0381x over previous
"""Trainium2 Bass kernel for nn_Attend_62534723830373.

Reference computation (note: q is UNUSED by the reference):
    scores = einsum('bhid,bhjd->bhij', k, v) * (1/sqrt(128))
    scores = causal_mask(scores)            # strictly-upper masked
    attn   = softmax(scores, axis=-1)
    out    = einsum('bhij,bhjd->bhid', attn, v)

Shapes: [b=2, h=16, s=2048, d=128] fp32. b*h = 32 head-slices sharded
4-per-core across 8 NeuronCores (data/head parallel, no collectives).

Per-head dataflow on one core (matmul chain in bf16, fp32 accumulate,
measured ~3e-3 scale-relative absmax vs the fp32 reference):
  - SWDGE cast-load K, V (fp32 HBM -> bf16 SBUF, natural layout), the
    first 4 row-blocks in their own chunk so compute starts early.
  - Just-in-time per 512-wide i-chunk: transpose the chunk's four
    128x128 blocks of K and V on the PE (bf16 transpose + DVE copy) to
    build KT[d, s] / VT[d, s], and assemble [V | ones] (129 cols).
  - For each i-chunk, j-block pairs share one 1024-wide (2-bank) PSUM
    score tile and ONE exp instruction (halves ACT's ~293ns fixed cost
    per instruction), emitted with one-pair lookahead so the PE always
    has score matmuls in flight:
      S^T[j, i] = (VT_blk).T @ KT_slice        (PE, contraction d)
      E = exp(SCALE * S^T)                     (ACT, PSUM -> SBUF bf16)
      diag block: E *= upper-tri 0/1 mask      (DVE)
      psum_o[i-blk] += E_slice.T @ [V_blk | 1] (PE, contraction j)
    The ones column makes column 128 of each accumulator the softmax
    denominator - numerator and denominator in one accumulation, and
    the denominator sees the same bf16 weights so quantization largely
    cancels in the normalization. Two accumulators share each PSUM
    bank; since matmul start=True clears has_written bank-wide, each
    bank is a single accumulation group (start only on the bank's first
    write, stop on its last; per-element has_written makes the second
    accumulator's first write an overwrite).
  - out = psum_o[:, 0:128] * (1 / psum_o[:, 128])  (DVE recip + mul),
    stored per i-chunk so the final DMA is small.

kernel(**inputs) takes FULL unsharded inputs and returns the FULL output.
"""

import numpy as np

B, H, S, D = 2, 16, 2048, 128
N_CORES = 8
HPC = (B * H) // N_CORES  # heads per core = 4
NB = S // 128             # 16 j/i blocks per head
NCH = S // 512            # 4 i-chunks per head
SCALE = 0.08838834764831845

_CACHED_NC = None


def _build_nc():
    import concourse.bass as bass
    import concourse.mybir as mybir
    import concourse.tile as tile
    from concourse import bacc
    from concourse.masks import make_identity, make_upper_triangular
    from contextlib import ExitStack

    f32 = mybir.dt.float32
    bf16 = mybir.dt.bfloat16
    Exp = mybir.ActivationFunctionType.Exp

    nc = bacc.Bacc("TRN2", num_devices=N_CORES, debug=False)
    kd = nc.dram_tensor("k", [HPC, S, D], f32, kind="ExternalInput")
    vd = nc.dram_tensor("v", [HPC, S, D], f32, kind="ExternalInput")
    od = nc.dram_tensor("out", [HPC, S, D], f32, kind="ExternalOutput")

    with tile.TileContext(nc) as tc, ExitStack() as ctx:
        const = ctx.enter_context(tc.tile_pool(name="const", bufs=1))
        loadp = ctx.enter_context(tc.tile_pool(name="load", bufs=2))
        ktp = ctx.enter_context(tc.tile_pool(name="kt", bufs=2))
        expp = ctx.enter_context(tc.tile_pool(name="expp", bufs=4))
        outp = ctx.enter_context(tc.tile_pool(name="outp", bufs=2))
        smallp = ctx.enter_context(tc.tile_pool(name="small", bufs=4))
        ps_pool = ctx.enter_context(tc.tile_pool(name="ps", bufs=2, space="PSUM"))
        pt_pool = ctx.enter_context(tc.tile_pool(name="pt", bufs=2, space="PSUM"))
        po_pool = ctx.enter_context(tc.tile_pool(name="po", bufs=2, space="PSUM"))

        trimask_f32 = const.tile([128, 128], f32, tag="trimask_f32")
        make_upper_triangular(nc, trimask_f32[:, :], val=1.0, diag=True)
        trimask = const.tile([128, 128], bf16, tag="trimask")
        nc.vector.tensor_copy(trimask[:, :], trimask_f32[:, :])
        onesf32 = const.tile([128, NB], f32, tag="onesf32")
        nc.gpsimd.memset(onesf32[:, :], 1.0)
        identbf = const.tile([128, 128], bf16, tag="identbf")
        make_identity(nc, identbf[:, :])
        # warmup exp so ACT's one-time table load happens during startup
        # instead of on the first real score tile's critical path
        warm = const.tile([128, 1], f32, tag="warm")
        nc.scalar.activation(warm[:, :], onesf32[:, 0:1], Exp, scale=SCALE)

        for h in range(HPC):
            # ---- loads: fp32 HBM -> bf16 SBUF (SWDGE cast), natural ----
            knat = loadp.tile([128, NB, 128], bf16, tag="knat")
            vnat = loadp.tile([128, NB, 128], bf16, tag="vnat")
            vones = loadp.tile([128, NB, 129], bf16, tag="vones")
            KT3 = ktp.tile([128, NB, 128], bf16, tag="KT")
            VT3 = ktp.tile([128, NB, 128], bf16, tag="VT")
            kview = kd.ap()[h].rearrange("(n p) d -> p n d", p=128)
            vview = vd.ap()[h].rearrange("(n p) d -> p n d", p=128)
            # first 4 blocks in their own chunk so chunk-0 compute can
            # start early (each SWDGE dma_start costs ~1.4us of Q7 issue,
            # so only two chunks per matrix)
            for c0, c1 in ((0, 4), (4, 16)):
                sl = slice(c0, c1)
                nc.gpsimd.dma_start(knat[:, sl, :], kview[:, sl, :])
                nc.gpsimd.dma_start(vnat[:, sl, :], vview[:, sl, :])
            KT = KT3.rearrange("p n d -> p (n d)")
            VT = VT3.rearrange("p n d -> p (n d)")

            out_sb = outp.tile([128, NB, 128], f32, tag="out_sb")

            # ---- main causal attention loop ----
            for ci in range(NCH):
                i0b = 4 * ci              # first i-block of chunk
                iend = (i0b + 4) * 128
                # just-in-time per chunk: [V | ones] columns and PE
                # transposes (+ DVE copies) for blocks 4ci..4ci+3; spreads
                # the transpose bursts across the head instead of one
                # stall-prone prologue burst
                slc = slice(i0b, i0b + 4)
                nc.vector.tensor_copy(vones[:, slc, 0:128], vnat[:, slc, :])
                nc.vector.tensor_copy(vones[:, slc, 128], onesf32[:, slc])
                for bn in range(i0b, i0b + 4):
                    pstk = pt_pool.tile(
                        [128, 128], bf16, tag="pt", name=f"pstk_{h}_{bn}"
                    )
                    nc.tensor.transpose(pstk[:, :], knat[:, bn, :], identbf[:, :])
                    nc.vector.tensor_copy(KT3[:, bn, :], pstk[:, :])
                    pstv = pt_pool.tile(
                        [128, 128], bf16, tag="pt", name=f"pstv_{h}_{bn}"
                    )
                    nc.tensor.transpose(pstv[:, :], vnat[:, bn, :], identbf[:, :])
                    nc.vector.tensor_copy(VT3[:, bn, :], pstv[:, :])
                po = [
                    po_pool.tile([128, 258], f32, tag="po", name=f"po_{h}_{ci}_{u}")
                    for u in range(2)
                ]

                def po_ap(bi):
                    u = bi - i0b
                    return po[u // 2][:, (u % 2) * 129 : (u % 2) * 129 + 129]

                # pairs are emitted with one-pair lookahead: pair k+1's
                # score matmuls + exp come before pair k's MM2s, so the PE
                # has work while the first MM2 of a chunk waits for the po
                # banks to be freed by the previous chunk's epilogue
                pending = None  # (bj_pair_state, ex) awaiting MM2 emission
                pairs = list(range(0, i0b + 4, 2)) + [None]
                for bja in pairs:
                    cur = None
                    if bja is not None:
                        bjb = bja + 1
                        ista = max(i0b, bja) * 128
                        istb_ = max(i0b, bjb) * 128
                        n1a = iend - ista
                        n1b = iend - istb_
                        ps = ps_pool.tile([128, 1024], f32, tag="ps")
                        nc.tensor.matmul(
                            ps[:, 0:n1a],
                            VT[:, bja * 128 : (bja + 1) * 128],
                            KT[:, ista:iend],
                            start=True,
                            stop=True,
                        )
                        nc.tensor.matmul(
                            ps[:, n1a : n1a + n1b],
                            VT[:, bjb * 128 : (bjb + 1) * 128],
                            KT[:, istb_:iend],
                            start=True,
                            stop=True,
                        )
                        ex = expp.tile([128, 1024], bf16, tag="ex")
                        nc.scalar.activation(
                            ex[:, 0 : n1a + n1b],
                            ps[:, 0 : n1a + n1b],
                            Exp,
                            scale=SCALE,
                        )
                        if bja >= i0b:
                            # diagonal blocks: zero j > i strict lower triangle
                            nc.vector.tensor_mul(
                                ex[:, 0:128], ex[:, 0:128], trimask[:, :]
                            )
                        if bjb >= i0b:
                            nc.vector.tensor_mul(
                                ex[:, n1a : n1a + 128],
                                ex[:, n1a : n1a + 128],
                                trimask[:, :],
                            )
                        cur = ((bja, ista, 0), (bjb, istb_, n1a), ex)
                    if pending is not None:
                        (pa, pb, pex) = pending
                        for bj, ist, off in (pa, pb):
                            for bi in range(ist // 128, i0b + 4):
                                c0 = off + bi * 128 - ist
                                nc.tensor.matmul(
                                    po_ap(bi),
                                    pex[:, c0 : c0 + 128],
                                    vones[:, bj, :],
                                    start=(bj == 0 and (bi - i0b) % 2 == 0),
                                    stop=(bj == bi and (bi - i0b) % 2 == 1),
                                    skip_group_check=True,
                                )
                    pending = cur
                for u in range(4):
                    bi = i0b + u
                    rc = smallp.tile([128, 1], f32, tag="rc")
                    nc.vector.reciprocal(rc[:, :], po_ap(bi)[:, 128:129])
                    nc.vector.tensor_scalar_mul(
                        out_sb[:, bi, :], po_ap(bi)[:, 0:128], rc[:, :]
                    )
                nc.sync.dma_start(
                    od.ap()[h].rearrange("(n p) d -> p n d", p=128)[
                        :, i0b : i0b + 4, :
                    ],
                    out_sb[:, i0b : i0b + 4, :],
                )

    nc.finalize()
    return nc


def _get_nc():
    global _CACHED_NC
    if _CACHED_NC is None:
        _CACHED_NC = _build_nc()
    return _CACHED_NC


def run_sharded(k, v, trace=False):
    """k, v: [B*H, S, D] fp32. Returns (out [B*H, S, D], BassKernelResults)."""
    from concourse import bass_utils

    nc = _get_nc()
    in_maps = [
        {
            "k": np.ascontiguousarray(k[c * HPC : (c + 1) * HPC]),
            "v": np.ascontiguousarray(v[c * HPC : (c + 1) * HPC]),
        }
        for c in range(N_CORES)
    ]
    res = bass_utils.run_bass_kernel_spmd(
        nc, in_maps, core_ids=list(range(N_CORES)), trace=trace
    )
    out = np.concatenate([res.results[c]["out"] for c in range(N_CORES)], axis=0)
    return out, res


def kernel(q, k, v):
    k = np.asarray(k, dtype=np.float32).reshape(B * H, S, D)
    v = np.asarray(v, dtype=np.float32).reshape(B * H, S, D)
    out, _ = run_sharded(k, v, trace=False)
    return out.reshape(B, H, S, D)
